# revision 51
# baseline (speedup 1.0000x reference)
"""Trainium2 Bass kernel for a leaky-integrate-fire (LIF) scan.

Reference computation (forward values only):
    v_t   = mem_{t-1} * 0.25 + x_t          (mem_0 carry = 0)
    s_t   = (v_t > 1.0) ? 1.0 : 0.0         (spike, the output)
    mem_t = (v_t <= 1.0) * v_t              (hard reset)

x: [T=32, B=64, N=16384] f32. Elementwise over (B, N), sequential over T.
Sharding: data-parallel over flattened B*N across 8 cores. Each core's slab
is laid out [P=128, T=32, F=1024] in DRAM.

Design (67.1 us/core on the TimelineSim cost model; baseline was 77.9):
- Resource walls per core: DMA 58.2 us (16.78 MB f32 load + 4.19 MB int8
  store at 360 B/ns, serialized on one DMA device) and the elementwise
  chain engines. DVE is the critical engine at ~60.9 us.
- Column split: DVE owns 784 cols with 2 fused scalar_tensor_tensor ops
  per step (v = mem*0.25 + x; mem = (v<=1)*v, 2.08 ns/col/step); Pool owns
  240 cols with 3 ops per step on a pre-scaled carry mem4 = 0.25*mem
  (v = mem4 + x; k4 = (v<=1)*0.25 via two-scalar tensor_scalar;
  mem4 = k4*v, 5.36 ns/col/step + launch overhead). W=240 found by sweep:
  larger W is better for raw balance but worsens the tile scheduler's
  ACT-queue ordering (vd-sign deferral) and the drain tail.
- DVE runs its ops as two independent 392-col streams interleaved, so the
  ~95 ns semaphore latency between dependent ops hides behind the other
  stream; all deps are 2 instructions back.
- Spikes off-chain on ACT: per-step Sign(v - 1) -> int8 {-1,0,1} per engine
  slice. For v in [0.5, 2), v - 1 is exact (Sterbenz), so sign(v-1) == 1
  <=> v > 1 exactly; host maps (raw == 1) -> 1.0f. int8 cuts store traffic
  4x. The final step's spikes are written by the chain engines themselves
  (is_gt -> 1/0 int8) so the drain skips the ACT handoff, and the last
  block's vd signs are split ACT [0:H] / Pool [H:D] so the closing store's
  dependency resolves ~1 us sooner; (raw == 1) decodes both encodings.
  Splitting more blocks or moving vg signs to Pool overloads Pool -- the
  split helps exactly where ACT's scheduler-deferred sign backlog meets
  the drain.
- DMA plumbing: ALL loads are emitted first on the SP ring as per-step
  0.5 MB pieces (4 KB descriptor lines keep full 360 B/ns) -- fine pieces
  always stay ahead of the ~1.9 us/step chain, where 2 MB block loads
  arrive in bursts and starve block boundaries. Stores ride the same ring
  after every load in queue order; the spike tiles (8 bufs) make every
  block's store independent. The first two steps' loads are split smaller
  so the chain starts ~3.5 us in. The last two blocks store per-step /
  in small pieces so each leaves as soon as its signs land and only the
  final 128 KB piece trails the chain.
- The final step skips the membrane update (never read).
- All on-device arithmetic is exactly reproducible fp32 (decay is a power
  of two, resets multiply by exactly 0.0/0.25), so the kernel matches the
  jax reference bitwise (0 mismatched elements).
- Measured busy/exec 67.2 us: DVE 60.9 (91%), DMA 58.2 (87%), Pool ~52,
  ACT 40.4. Rejected: PE matmul offload (float32r is rejected by the axon
  runtime; plain f32 matmul is 4 cycles/row -- slower per column than
  DVE), bf16 anywhere (spike flips blow the 2e-2 gate), bit-packed output
  (+1 op/elem on saturated engines costs more than the 10 us of DMA it
  saves), time-chunked parallel scan (engines are throughput-bound, not
  latency-bound, and warmup duplicates work).
"""

import numpy as np

T = 32
B = 64
N = 16384
NCORES = 8
P = 128                      # SBUF partitions
F = (B // NCORES) * N // P   # 1024 free-dim columns per step per core
TB = 4                       # timesteps per DMA block (2 MiB loads)
SB = 2                       # timesteps per Sign batch (ACT fixed-cost amortize)
W = 240                      # columns whose chain runs on the Pool engine
DECAY = 0.25
VTH = 1.0

_CACHE = {}


def _build_program():
    import concourse.bacc as bacc
    import concourse.tile as tile
    from concourse import mybir

    nc = bacc.Bacc(
        target_bir_lowering=False,
        debug=False,
        enable_asserts=False,
        num_devices=NCORES,
    )
    f32 = mybir.dt.float32
    i8 = mybir.dt.int8
    Alu = mybir.AluOpType
    Act = mybir.ActivationFunctionType
    D = F - W
    H = D // 2               # DVE stream size
    NBLK = T // TB

    x_d = nc.dram_tensor("x", [P, T, F], f32, kind="ExternalInput").ap()
    o_d = nc.dram_tensor("out", [P, T, F], i8, kind="ExternalOutput").ap()

    with tile.TileContext(nc) as tc:
        with (
            tc.tile_pool(name="xp", bufs=4) as xpool,
            tc.tile_pool(name="sp", bufs=8) as spool,
            tc.tile_pool(name="vp", bufs=8) as vpool,
            tc.tile_pool(name="kp", bufs=3) as kpool,
            tc.tile_pool(name="mp", bufs=1) as mpool,
        ):
            mem = mpool.tile([P, D], f32)     # DVE-owned membrane carry
            nc.vector.memset(mem[:], 0.0)
            nbias = mpool.tile([P, 1], f32)   # per-partition bias = -VTH
            nc.vector.memset(nbias[:], -VTH)
            mem4 = mpool.tile([P, W], f32)    # Pool-owned carry, pre-scaled 0.25x
            nc.gpsimd.memset(mem4[:], 0.0)
            # Phase 1: emit every load up front so the SP queue is pure
            # loads followed by pure stores -- a store waiting on spike
            # tiles then never delays load issuance, and the ACT queue
            # carries only Sign work so signs track the chain closely.
            xts = []
            for blk in range(NBLK):
                xt = xpool.tile([P, TB, F], f32)
                if blk == 0:
                    # the first load is split by DVE stream so the chain
                    # starts as early as possible
                    nc.sync.dma_start(out=xt[:, 0:1, :H], in_=x_d[:, 0:1, :H])
                    nc.sync.dma_start(out=xt[:, 0:1, H:D], in_=x_d[:, 0:1, H:D])
                    nc.sync.dma_start(out=xt[:, 1:2, :D], in_=x_d[:, 1:2, :D])
                    nc.sync.dma_start(out=xt[:, 0:1, D:], in_=x_d[:, 0:1, D:])
                    nc.sync.dma_start(out=xt[:, 1:2, D:], in_=x_d[:, 1:2, D:])
                    jstart = 2
                else:
                    jstart = 0
                # per-step 0.5 MB pieces always stay ahead of the chain
                # (1456 ns/step DMA vs ~1800 ns/step compute)
                for j in range(jstart, TB):
                    nc.sync.dma_start(
                        out=xt[:, j:j + 1],
                        in_=x_d[:, blk * TB + j:blk * TB + j + 1, :])
                xts.append(xt)
            for blk in range(NBLK):
                xt = xts[blk]
                st = spool.tile([P, TB, F], i8)
                for g in range(TB // SB):
                    # separate per-engine v tiles so the DVE and Pool chains
                    # never share a tile; tiles rotate so the off-chain
                    # compare can overlap the chain of later steps.
                    vd = vpool.tile([P, SB, D], f32, name="vd")
                    vg = vpool.tile([P, SB, W], f32, name="vg")
                    for jj in range(SB):
                        j = g * SB + jj
                        last = blk == NBLK - 1 and j == T - 1
                        # DVE chain, 2 interleaved independent streams
                        for a, b in ((0, H), (H, D)):
                            nc.vector.scalar_tensor_tensor(
                                out=vd[:, jj, a:b], in0=mem[:, a:b],
                                scalar=DECAY, in1=xt[:, j, a:b],
                                op0=Alu.mult, op1=Alu.add,
                            )
                        if not last:  # final membrane is never read
                            for a, b in ((0, H), (H, D)):
                                nc.vector.scalar_tensor_tensor(
                                    out=mem[:, a:b], in0=vd[:, jj, a:b],
                                    scalar=VTH, in1=vd[:, jj, a:b],
                                    op0=Alu.is_le, op1=Alu.mult,
                                )
                        # Pool chain, columns [D, F): 3 ops per step on the
                        # pre-scaled carry (mem4 == 0.25*mem exactly)
                        nc.gpsimd.tensor_tensor(
                            out=vg[:, jj, :], in0=mem4[:], in1=xt[:, j, D:],
                            op=Alu.add,
                        )
                        if not last:
                            k4 = kpool.tile([P, W], f32)
                            nc.gpsimd.tensor_scalar(
                                out=k4[:], in0=vg[:, jj, :], scalar1=VTH,
                                scalar2=DECAY, op0=Alu.is_le, op1=Alu.mult,
                            )
                            nc.gpsimd.tensor_tensor(
                                out=mem4[:], in0=k4[:], in1=vg[:, jj, :],
                                op=Alu.mult,
                            )
                        if last:
                            # final step: each chain engine writes its own
                            # spikes (is_gt -> int8 1/0; host decodes raw==1)
                            # so the drain doesn't wait on an ACT handoff
                            nc.vector.tensor_scalar(
                                out=st[:, j, :D], in0=vd[:, jj, :],
                                scalar1=VTH, scalar2=None, op0=Alu.is_gt)
                            nc.gpsimd.tensor_scalar(
                                out=st[:, j, D:], in0=vg[:, jj, :],
                                scalar1=VTH, scalar2=None, op0=Alu.is_gt)
                        elif blk == NBLK - 1:
                            # late steps: split the big vd sign across ACT
                            # and the (slack) Pool engine so the last store's
                            # dependency resolves sooner
                            nc.scalar.activation(
                                st[:, j:j + 1, :H], vd[:, jj:jj + 1, :H],
                                Act.Sign, bias=nbias[:])
                            nc.gpsimd.tensor_scalar(
                                out=st[:, j, H:D], in0=vd[:, jj, H:],
                                scalar1=VTH, scalar2=None, op0=Alu.is_gt)
                            nc.scalar.activation(
                                st[:, j:j + 1, D:], vg[:, jj:jj + 1],
                                Act.Sign, bias=nbias[:])
                        else:
                            # per-step signs: small pieces track the chain
                            # closely so no sign backlog remains at the drain
                            nc.scalar.activation(
                                st[:, j:j + 1, :D], vd[:, jj:jj + 1],
                                Act.Sign, bias=nbias[:])
                            nc.scalar.activation(
                                st[:, j:j + 1, D:], vg[:, jj:jj + 1],
                                Act.Sign, bias=nbias[:])
                if blk == NBLK - 1:
                    # fine-grained closing stores: each piece leaves as soon
                    # as its signs land, so only the DVE-gated final 128 KB
                    # piece remains after the chain ends
                    nc.sync.dma_start(
                        out=o_d[:, blk * TB:blk * TB + 1, :], in_=st[:, :1])
                    nc.sync.dma_start(
                        out=o_d[:, blk * TB + 1:blk * TB + 2, :], in_=st[:, 1:2])
                    nc.sync.dma_start(
                        out=o_d[:, blk * TB + 2:blk * TB + 3, :], in_=st[:, 2:3])
                    nc.sync.dma_start(
                        out=o_d[:, blk * TB + 3:(blk + 1) * TB, :], in_=st[:, 3:])
                elif blk == NBLK - 2:
                    # split so the piece gated by late-deferred signs is small
                    nc.sync.dma_start(
                        out=o_d[:, blk * TB:blk * TB + 2, :], in_=st[:, :2])
                    nc.sync.dma_start(
                        out=o_d[:, blk * TB + 2:(blk + 1) * TB, :], in_=st[:, 2:])
                else:
                    nc.sync.dma_start(
                        out=o_d[:, blk * TB:(blk + 1) * TB, :], in_=st[:])
    nc.compile()
    return nc


def _get_nc():
    if "nc" not in _CACHE:
        _CACHE["nc"] = _build_program()
    return _CACHE["nc"]


def _get_runner():
    """Cache one jitted SPMD executable (same lowering as
    bass_utils.run_bass_kernel_spmd's axon path, which builds a fresh
    jax.jit closure per call and would recompile every time)."""
    if "runner" in _CACHE:
        return _CACHE["runner"]

    import jax
    from jax.sharding import Mesh, PartitionSpec
    from jax.experimental.shard_map import shard_map
    from concourse import bass2jax

    nc = _get_nc()
    bass2jax.install_neuronx_cc_hook()

    # operand order: real inputs, donated output buffers, partition_id last
    in_names = ("x", "out", "partition_id")
    out_names = ("out",)
    out_avals = (jax.core.ShapedArray((P, T, F), np.int8),)

    def _body(*args):
        outs = bass2jax._bass_exec_p.bind(
            *args,
            bass2jax.partition_id_tensor(),
            out_avals=out_avals,
            in_names=in_names,
            out_names=out_names,
            lowering_input_output_aliases=(),
            sim_require_finite=True,
            sim_require_nnan=True,
            nc=nc,
        )
        return tuple(outs)

    devices = jax.devices()[:NCORES]
    mesh = Mesh(np.asarray(devices), ("core",))
    sharded = jax.jit(
        shard_map(
            _body,
            mesh=mesh,
            in_specs=(PartitionSpec("core"),) * 2,
            out_specs=(PartitionSpec("core"),),
            check_rep=False,
        ),
        donate_argnums=(1,),
        keep_unused=True,
    )
    _CACHE["runner"] = sharded
    return sharded


def _run_sharded(x_concat):
    """x_concat: [NCORES*P, T, F] host array, core k's slab at rows k*P:(k+1)*P."""
    runner = _get_runner()
    zeros = np.zeros((NCORES * P, T, F), np.int8)
    (out,) = runner(x_concat, zeros)
    return np.asarray(out)


def kernel(x):
    x = np.asarray(x, dtype=np.float32)
    assert x.shape == (T, B, N), x.shape
    # [T, B, N] -> [T, 8, P, F] -> per-core [8, P, T, F] -> concat on axis 0
    x_concat = np.ascontiguousarray(
        x.reshape(T, NCORES, P, F).transpose(1, 2, 0, 3)
    ).reshape(NCORES * P, T, F)
    out = _run_sharded(x_concat)
    # [8*P, T, F] -> [8, P, T, F] -> [T, 8, P, F] -> [T, B, N]
    out = np.ascontiguousarray(
        out.reshape(NCORES, P, T, F).transpose(2, 0, 1, 3)
    ).reshape(T, B, N)
    # raw == 1 <=> v > VTH; exact 0.0/1.0 reconstruction
    return (out == 1).astype(np.float32)


# revision 52
# speedup vs baseline: 1.0166x; 1.0166x over previous
"""Trainium2 Bass kernel for a leaky-integrate-fire (LIF) scan.

Reference computation (forward values only):
    v_t   = mem_{t-1} * 0.25 + x_t          (mem_0 carry = 0)
    s_t   = (v_t > 1.0) ? 1.0 : 0.0         (spike, the output)
    mem_t = (v_t <= 1.0) * v_t              (hard reset)

x: [T=32, B=64, N=16384] f32. Elementwise over (B, N), sequential over T.
Sharding: data-parallel over flattened B*N across 8 cores. Each core's slab
is laid out [P=128, T=32, F=1024] in DRAM.

Design (67.1 us/core on the TimelineSim cost model; baseline was 77.9):
- Resource walls per core: DMA 58.2 us (16.78 MB f32 load + 4.19 MB int8
  store at 360 B/ns, serialized on one DMA device) and the elementwise
  chain engines. DVE is the critical engine at ~60.9 us.
- Column split: DVE owns 784 cols with 2 fused scalar_tensor_tensor ops
  per step (v = mem*0.25 + x; mem = (v<=1)*v, 2.08 ns/col/step); Pool owns
  240 cols with 3 ops per step on a pre-scaled carry mem4 = 0.25*mem
  (v = mem4 + x; k4 = (v<=1)*0.25 via two-scalar tensor_scalar;
  mem4 = k4*v, 5.36 ns/col/step + launch overhead). W=240 found by sweep:
  larger W is better for raw balance but worsens the tile scheduler's
  ACT-queue ordering (vd-sign deferral) and the drain tail.
- DVE runs its ops as two independent 392-col streams interleaved, so the
  ~95 ns semaphore latency between dependent ops hides behind the other
  stream; all deps are 2 instructions back.
- Spikes off-chain on ACT: per-step Sign(v - 1) -> int8 {-1,0,1} per engine
  slice. For v in [0.5, 2), v - 1 is exact (Sterbenz), so sign(v-1) == 1
  <=> v > 1 exactly; host maps (raw == 1) -> 1.0f. int8 cuts store traffic
  4x. The final step's spikes are written by the chain engines themselves
  (is_gt -> 1/0 int8) so the drain skips the ACT handoff, and the last
  block's vd signs are split ACT [0:H] / Pool [H:D] so the closing store's
  dependency resolves ~1 us sooner; (raw == 1) decodes both encodings.
  Splitting more blocks or moving vg signs to Pool overloads Pool -- the
  split helps exactly where ACT's scheduler-deferred sign backlog meets
  the drain.
- DMA plumbing: ALL loads are emitted first on the SP ring as per-step
  0.5 MB pieces (4 KB descriptor lines keep full 360 B/ns) -- fine pieces
  always stay ahead of the ~1.9 us/step chain, where 2 MB block loads
  arrive in bursts and starve block boundaries. Stores ride the same ring
  after every load in queue order; the spike tiles (8 bufs) make every
  block's store independent. The first two steps' loads are split smaller
  so the chain starts ~3.5 us in. The last two blocks store per-step /
  in small pieces so each leaves as soon as its signs land and only the
  final 128 KB piece trails the chain.
- The final step skips the membrane update (never read).
- All on-device arithmetic is exactly reproducible fp32 (decay is a power
  of two, resets multiply by exactly 0.0/0.25), so the kernel matches the
  jax reference bitwise (0 mismatched elements).
- Measured busy/exec 67.2 us: DVE 60.9 (91%), DMA 58.2 (87%), Pool ~52,
  ACT 40.4. Rejected: PE matmul offload (float32r is rejected by the axon
  runtime; plain f32 matmul is 4 cycles/row -- slower per column than
  DVE), bf16 anywhere (spike flips blow the 2e-2 gate), bit-packed output
  (+1 op/elem on saturated engines costs more than the 10 us of DMA it
  saves), time-chunked parallel scan (engines are throughput-bound, not
  latency-bound, and warmup duplicates work).
"""

import numpy as np

T = 32
B = 64
N = 16384
NCORES = 8
P = 128                      # SBUF partitions
F = (B // NCORES) * N // P   # 1024 free-dim columns per step per core
TB = 4                       # timesteps per DMA block (2 MiB loads)
SB = 2                       # timesteps per Sign batch (ACT fixed-cost amortize)
W = 240                      # columns whose chain runs on the Pool engine
DECAY = 0.25
VTH = 1.0

_CACHE = {}


def _build_program():
    import concourse.bacc as bacc
    import concourse.tile as tile
    from concourse import mybir

    nc = bacc.Bacc(
        target_bir_lowering=False,
        debug=False,
        enable_asserts=False,
        num_devices=NCORES,
    )
    f32 = mybir.dt.float32
    i8 = mybir.dt.int8
    Alu = mybir.AluOpType
    Act = mybir.ActivationFunctionType
    D = F - W
    H = D // 2               # DVE stream size
    NBLK = T // TB

    x_d = nc.dram_tensor("x", [P, T, F], f32, kind="ExternalInput").ap()
    o_d = nc.dram_tensor("out", [P, T, F], i8, kind="ExternalOutput").ap()

    with tile.TileContext(nc) as tc:
        with (
            tc.tile_pool(name="xp", bufs=4) as xpool,
            tc.tile_pool(name="sp", bufs=8) as spool,
            tc.tile_pool(name="vp", bufs=8) as vpool,
            tc.tile_pool(name="kp", bufs=3) as kpool,
            tc.tile_pool(name="mp", bufs=1) as mpool,
        ):
            mem = mpool.tile([P, D], f32)     # DVE-owned membrane carry
            nc.vector.memset(mem[:], 0.0)
            nbias = mpool.tile([P, 1], f32)   # per-partition bias = -VTH
            nc.vector.memset(nbias[:], -VTH)
            mem4 = mpool.tile([P, W], f32)    # Pool-owned carry, pre-scaled 0.25x
            nc.gpsimd.memset(mem4[:], 0.0)
            # Phase 1: emit every load up front so the SP queue is pure
            # loads followed by pure stores -- a store waiting on spike
            # tiles then never delays load issuance, and the ACT queue
            # carries only Sign work so signs track the chain closely.
            xts = []
            for blk in range(NBLK):
                xt = xpool.tile([P, TB, F], f32)
                if blk == 0:
                    # the first load is split by DVE stream so the chain
                    # starts as early as possible
                    nc.sync.dma_start(out=xt[:, 0:1, :H], in_=x_d[:, 0:1, :H])
                    nc.sync.dma_start(out=xt[:, 0:1, H:D], in_=x_d[:, 0:1, H:D])
                    nc.sync.dma_start(out=xt[:, 0:1, D:], in_=x_d[:, 0:1, D:])
                    nc.sync.dma_start(out=xt[:, 1:2, :D], in_=x_d[:, 1:2, :D])
                    nc.sync.dma_start(out=xt[:, 1:2, D:], in_=x_d[:, 1:2, D:])
                    jstart = 2
                else:
                    jstart = 0
                # per-step 0.5 MB pieces always stay ahead of the chain
                # (1456 ns/step DMA vs ~1800 ns/step compute)
                for j in range(jstart, TB):
                    nc.sync.dma_start(
                        out=xt[:, j:j + 1],
                        in_=x_d[:, blk * TB + j:blk * TB + j + 1, :])
                xts.append(xt)
            for blk in range(NBLK):
                xt = xts[blk]
                st = spool.tile([P, TB, F], i8)
                for g in range(TB // SB):
                    # separate per-engine v tiles so the DVE and Pool chains
                    # never share a tile; tiles rotate so the off-chain
                    # compare can overlap the chain of later steps.
                    vd = vpool.tile([P, SB, D], f32, name="vd")
                    vg = vpool.tile([P, SB, W], f32, name="vg")
                    for jj in range(SB):
                        j = g * SB + jj
                        last = blk == NBLK - 1 and j == T - 1
                        # DVE chain, 2 interleaved independent streams
                        for a, b in ((0, H), (H, D)):
                            nc.vector.scalar_tensor_tensor(
                                out=vd[:, jj, a:b], in0=mem[:, a:b],
                                scalar=DECAY, in1=xt[:, j, a:b],
                                op0=Alu.mult, op1=Alu.add,
                            )
                        if not last:  # final membrane is never read
                            for a, b in ((0, H), (H, D)):
                                nc.vector.scalar_tensor_tensor(
                                    out=mem[:, a:b], in0=vd[:, jj, a:b],
                                    scalar=VTH, in1=vd[:, jj, a:b],
                                    op0=Alu.is_le, op1=Alu.mult,
                                )
                        # Pool chain, columns [D, F): 3 ops per step on the
                        # pre-scaled carry (mem4 == 0.25*mem exactly)
                        nc.gpsimd.tensor_tensor(
                            out=vg[:, jj, :], in0=mem4[:], in1=xt[:, j, D:],
                            op=Alu.add,
                        )
                        if not last:
                            k4 = kpool.tile([P, W], f32)
                            nc.gpsimd.tensor_scalar(
                                out=k4[:], in0=vg[:, jj, :], scalar1=VTH,
                                scalar2=DECAY, op0=Alu.is_le, op1=Alu.mult,
                            )
                            nc.gpsimd.tensor_tensor(
                                out=mem4[:], in0=k4[:], in1=vg[:, jj, :],
                                op=Alu.mult,
                            )
                        if last:
                            # final step: each chain engine writes its own
                            # spikes (is_gt -> int8 1/0; host decodes raw==1)
                            # so the drain doesn't wait on an ACT handoff
                            nc.vector.tensor_scalar(
                                out=st[:, j, :D], in0=vd[:, jj, :],
                                scalar1=VTH, scalar2=None, op0=Alu.is_gt)
                            nc.gpsimd.tensor_scalar(
                                out=st[:, j, D:], in0=vg[:, jj, :],
                                scalar1=VTH, scalar2=None, op0=Alu.is_gt)
                        elif blk == NBLK - 1:
                            # late steps: split the big vd sign across ACT
                            # and the (slack) Pool engine so the last store's
                            # dependency resolves sooner
                            nc.scalar.activation(
                                st[:, j:j + 1, :H], vd[:, jj:jj + 1, :H],
                                Act.Sign, bias=nbias[:])
                            nc.gpsimd.tensor_scalar(
                                out=st[:, j, H:D], in0=vd[:, jj, H:],
                                scalar1=VTH, scalar2=None, op0=Alu.is_gt)
                            nc.scalar.activation(
                                st[:, j:j + 1, D:], vg[:, jj:jj + 1],
                                Act.Sign, bias=nbias[:])
                        else:
                            # per-step signs: small pieces track the chain
                            # closely so no sign backlog remains at the drain
                            nc.scalar.activation(
                                st[:, j:j + 1, :D], vd[:, jj:jj + 1],
                                Act.Sign, bias=nbias[:])
                            nc.scalar.activation(
                                st[:, j:j + 1, D:], vg[:, jj:jj + 1],
                                Act.Sign, bias=nbias[:])
                if blk == NBLK - 1:
                    # fine-grained closing stores: each piece leaves as soon
                    # as its signs land, so only the DVE-gated final 128 KB
                    # piece remains after the chain ends
                    nc.sync.dma_start(
                        out=o_d[:, blk * TB:blk * TB + 1, :], in_=st[:, :1])
                    nc.sync.dma_start(
                        out=o_d[:, blk * TB + 1:blk * TB + 2, :], in_=st[:, 1:2])
                    nc.sync.dma_start(
                        out=o_d[:, blk * TB + 2:blk * TB + 3, :], in_=st[:, 2:3])
                    nc.sync.dma_start(
                        out=o_d[:, blk * TB + 3:(blk + 1) * TB, :], in_=st[:, 3:])
                elif blk == NBLK - 2:
                    # split so the piece gated by late-deferred signs is small
                    nc.sync.dma_start(
                        out=o_d[:, blk * TB:blk * TB + 2, :], in_=st[:, :2])
                    nc.sync.dma_start(
                        out=o_d[:, blk * TB + 2:(blk + 1) * TB, :], in_=st[:, 2:])
                else:
                    nc.sync.dma_start(
                        out=o_d[:, blk * TB:(blk + 1) * TB, :], in_=st[:])
    nc.compile()
    return nc


def _get_nc():
    if "nc" not in _CACHE:
        _CACHE["nc"] = _build_program()
    return _CACHE["nc"]


def _get_runner():
    """Cache one jitted SPMD executable (same lowering as
    bass_utils.run_bass_kernel_spmd's axon path, which builds a fresh
    jax.jit closure per call and would recompile every time)."""
    if "runner" in _CACHE:
        return _CACHE["runner"]

    import jax
    from jax.sharding import Mesh, PartitionSpec
    from jax.experimental.shard_map import shard_map
    from concourse import bass2jax

    nc = _get_nc()
    bass2jax.install_neuronx_cc_hook()

    # operand order: real inputs, donated output buffers, partition_id last
    in_names = ("x", "out", "partition_id")
    out_names = ("out",)
    out_avals = (jax.core.ShapedArray((P, T, F), np.int8),)

    def _body(*args):
        outs = bass2jax._bass_exec_p.bind(
            *args,
            bass2jax.partition_id_tensor(),
            out_avals=out_avals,
            in_names=in_names,
            out_names=out_names,
            lowering_input_output_aliases=(),
            sim_require_finite=True,
            sim_require_nnan=True,
            nc=nc,
        )
        return tuple(outs)

    devices = jax.devices()[:NCORES]
    mesh = Mesh(np.asarray(devices), ("core",))
    sharded = jax.jit(
        shard_map(
            _body,
            mesh=mesh,
            in_specs=(PartitionSpec("core"),) * 2,
            out_specs=(PartitionSpec("core"),),
            check_rep=False,
        ),
        donate_argnums=(1,),
        keep_unused=True,
    )
    _CACHE["runner"] = sharded
    return sharded


def _run_sharded(x_concat):
    """x_concat: [NCORES*P, T, F] host array, core k's slab at rows k*P:(k+1)*P."""
    runner = _get_runner()
    zeros = np.zeros((NCORES * P, T, F), np.int8)
    (out,) = runner(x_concat, zeros)
    return np.asarray(out)


def kernel(x):
    x = np.asarray(x, dtype=np.float32)
    assert x.shape == (T, B, N), x.shape
    # [T, B, N] -> [T, 8, P, F] -> per-core [8, P, T, F] -> concat on axis 0
    x_concat = np.ascontiguousarray(
        x.reshape(T, NCORES, P, F).transpose(1, 2, 0, 3)
    ).reshape(NCORES * P, T, F)
    out = _run_sharded(x_concat)
    # [8*P, T, F] -> [8, P, T, F] -> [T, 8, P, F] -> [T, B, N]
    out = np.ascontiguousarray(
        out.reshape(NCORES, P, T, F).transpose(2, 0, 1, 3)
    ).reshape(T, B, N)
    # raw == 1 <=> v > VTH; exact 0.0/1.0 reconstruction
    return (out == 1).astype(np.float32)
